# revision 22
# baseline (speedup 1.0000x reference)
"""Fused BN+ReLU -> 1x1 conv -> BN+ReLU -> 3x3 conv -> concat kernel for TRN2.

Data-parallel over batch: 32 images are sharded 4-per-core across 8 NeuronCores.
BN params / conv weights are folded host-side and replicated.

Per-core structure (per image):
  - x image [256, 3136] is DMA'd to SBUF [128p, 2blk, 3136] in two row-chunks.
  - BN1+ReLU on the scalar engine (per-partition scale/bias), rounding to
    bf16 so the PE runs matmuls at full rate (4x faster than fp32) with
    weight loads hidden by the PE reorder window.
  - conv1 (1x1, 256->128): 2 accumulating matmuls per 8-row chunk into PSUM.
  - BN2+ReLU evicts PSUM into a zero-padded [128, 58, 58] image.
  - conv2 (3x3, 128->32): 9 shifted accumulating matmuls per 8-row chunk;
    DVE adds the conv2 bias into a staging tile; one DMA out per image.
The SP HWDGE stream carries only the x loads; params ride the ACT HWDGE and
conv2 outputs the Pool SWDGE so nothing head-of-line-blocks the loads. The
concat's verbatim x channels (0..255) are assembled on the host during the
gather/unshard step from the host-resident input, so device HBM traffic per
core is 12.85 MB in + 1.6 MB out.
"""

import numpy as np

EPS = 1e-5

N_CORES = 8
N_PER_CORE = 4          # 32 images / 8 cores
C_IN = 256
C_MID = 128
C_OUT = 32
H = W = 56
S = H * W               # 3136
PW = W + 2              # padded row width 58
PH = H + 2
ROWS = 8                # image rows per matmul chunk
NCHUNK = H // ROWS      # 7
CHUNK_N = ROWS * W      # 448 <= 512 (fp32 PSUM bank limit)
# x-load / BN1 pieces (in conv chunks): a small first piece so the first
# matmul starts ~5us in, then two larger ones
PIECES = [(0, 2), (2, 4), (4, 7)]    # chunk ranges per piece
PIECE_SL = [(lo * CHUNK_N, hi * CHUNK_N) for lo, hi in PIECES]

_CACHE = {}


def _build_program():
    if "nc" in _CACHE:
        return _CACHE["nc"]

    from contextlib import ExitStack

    import concourse.bacc as bacc
    import concourse.tile as tile
    from concourse import mybir

    f32 = mybir.dt.float32
    bf16 = mybir.dt.bfloat16  # matmul operands: full-rate PE + hidden LDWEIGHTS
    Relu = mybir.ActivationFunctionType.Relu

    nc = bacc.Bacc("TRN2", target_bir_lowering=False, debug=False,
                   num_devices=N_CORES)

    x_d = nc.declare_dram_parameter("x", [N_PER_CORE, C_IN, H, W], f32,
                                    isOutput=False)
    w1t_d = nc.declare_dram_parameter("w1t", [128, 2, C_MID], bf16,
                                      isOutput=False)
    w2t_d = nc.declare_dram_parameter("w2t", [128, 9, C_OUT], bf16,
                                      isOutput=False)
    bnp_d = nc.declare_dram_parameter("bnp", [128, 8], f32, isOutput=False)
    out_d = nc.declare_dram_parameter("out", [N_PER_CORE, C_OUT, H, W],
                                      f32, isOutput=True)

    with tile.TileContext(nc) as tc, ExitStack() as ctx:
        const = ctx.enter_context(tc.tile_pool(name="const", bufs=1))
        raw_p = ctx.enter_context(tc.tile_pool(name="raw", bufs=3))
        bn_p = ctx.enter_context(tc.tile_pool(name="bn", bufs=2))
        mid_p = ctx.enter_context(tc.tile_pool(name="mid", bufs=2))
        stage_p = ctx.enter_context(tc.tile_pool(name="stage", bufs=2))
        ps1_p = ctx.enter_context(tc.tile_pool(name="ps1", bufs=4, space="PSUM"))
        ps2_p = ctx.enter_context(tc.tile_pool(name="ps2", bufs=4, space="PSUM"))

        # params via the ACT-engine HWDGE: their DMA-resource requests land
        # ahead of the big x loads (tiny transfers), and they stay off the SP
        # HWDGE stream that feeds the x loads / copies
        bnp_s = const.tile([128, 8], f32)
        nc.scalar.dma_start(out=bnp_s[:], in_=bnp_d[:])
        w1t_s = const.tile([128, 2, C_MID], bf16)
        nc.scalar.dma_start(out=w1t_s[:], in_=w1t_d[:])
        w2t_s = const.tile([128, 9, C_OUT], bf16)
        nc.scalar.dma_start(out=w2t_s[:], in_=w2t_d[:])

        # tiny warm-up activation: hoists the ACT function-table load off the
        # first image's critical path
        warm = const.tile([128, 1], f32)
        nc.vector.memset(warm[:], 0.0)
        nc.scalar.activation(warm[:], warm[:], Relu)

        raws = {}

        def load_x(n):
            raws[n] = raw_p.tile([128, 2, S], f32, tag="raw", name=f"raw{n}")
            x_img = x_d[n].rearrange("(b p) h w -> p b (h w)", p=128)
            for lo, hi in PIECE_SL:
                nc.sync.dma_start(out=raws[n][:, :, lo:hi],
                                  in_=x_img[:, :, lo:hi])

        # SP HWDGE stream: all loads, back to back. raw bufs=3 means load 3
        # starts once image 0's BN1 has consumed its buffer.
        load_x(0)
        load_x(1)
        load_x(2)

        for n in range(N_PER_CORE):
            raw = raws.pop(n)
            if n + 3 < N_PER_CORE:
                load_x(n + 3)

            # BN1 + ReLU on DVE as z = s1*max(x, -t1/s1) (valid since s1>0);
            # the missing +t1 is folded into BN2's bias host-side (it passes
            # through the 1x1 conv as a constant). One tile per load piece so
            # conv1 chunk c only waits on its piece's two ops, and the DVE
            # (idle at start) takes BN1 off the ACT critical path.
            bns = []
            for pi, (lo, hi) in enumerate(PIECE_SL):
                bnt = bn_p.tile([128, 2, hi - lo], bf16, tag=f"bn{pi}",
                                name=f"bn{pi}_{n}")
                bns.append(bnt)
                for b in range(2):
                    nc.vector.tensor_scalar(
                        bnt[:, b], raw[:, b, lo:hi],
                        bnp_s[:, 2 + b:3 + b], bnp_s[:, b:b + 1],
                        mybir.AluOpType.max, mybir.AluOpType.mult)

            # zero-padded conv1 output image [128, 58, 58]
            mid = mid_p.tile([128, PH, PW], bf16)
            nc.gpsimd.memset(mid[:, 0, :], 0.0)
            nc.gpsimd.memset(mid[:, PH - 1, :], 0.0)
            nc.gpsimd.memset(mid[:, 1:PH - 1, 0:1], 0.0)
            nc.gpsimd.memset(mid[:, 1:PH - 1, PW - 1:PW], 0.0)

            # conv1 (1x1, 256->128) + BN2 + ReLU, 8 rows at a time
            for c in range(NCHUNK):
                pi = next(i for i, (lo, hi) in enumerate(PIECES)
                          if lo <= c < hi)
                bnc = bns[pi]
                off = (c - PIECES[pi][0]) * CHUNK_N
                ps1 = ps1_p.tile([128, ROWS, W], f32)
                for b in range(2):
                    nc.tensor.matmul(ps1[:], w1t_s[:, b],
                                     bnc[:, b, off:off + CHUNK_N],
                                     start=(b == 0), stop=(b == 1))
                nc.scalar.activation(mid[:, 1 + ROWS * c:1 + ROWS * (c + 1),
                                         1:1 + W],
                                     ps1[:], Relu,
                                     bias=bnp_s[:, 5:6], scale=bnp_s[:, 4:5])

            # conv2 (3x3, 128->32) as 9 shifted accumulating matmuls
            stage = stage_p.tile([C_OUT, S], f32)
            for c in range(NCHUNK):
                ps2 = ps2_p.tile([C_OUT, CHUNK_N], f32)
                for t in range(9):
                    dy, dx = divmod(t, 3)
                    rhs = mid[:, ROWS * c + dy:ROWS * c + dy + ROWS,
                              dx:dx + W]
                    nc.tensor.matmul(ps2[:], w2t_s[:, t], rhs,
                                     start=(t == 0), stop=(t == 8))
                # + conv2 bias, PSUM -> SBUF staging
                nc.vector.tensor_scalar_add(
                    stage[:, c * CHUNK_N:(c + 1) * CHUNK_N], ps2[:],
                    bnp_s[0:C_OUT, 6:7])

            # SWDGE on the (mostly idle) Pool engine so this DMA, which waits
            # on conv2, never head-of-line-blocks the SP x-load stream
            out_comp = out_d[n].rearrange("c h w -> c (h w)")
            nc.gpsimd.dma_start(out=out_comp, in_=stage[:])

    nc.compile()
    _CACHE["nc"] = nc
    return nc


def _fold_params(conv1_w, conv1_b, conv2_w, conv2_b,
                 bn1_gamma, bn1_beta, bn1_mean, bn1_var,
                 bn2_gamma, bn2_beta, bn2_mean, bn2_var):
    s1 = bn1_gamma / np.sqrt(bn1_var + EPS)                     # [256]
    t1 = bn1_beta - bn1_mean * s1                               # [256]
    s2 = bn2_gamma / np.sqrt(bn2_var + EPS)                     # [128]
    # conv1 bias, plus BN1's +t1 pushed through the 1x1 conv (see kernel)
    w1 = conv1_w[:, :, 0, 0]                                    # [128, 256]
    t2 = (bn2_beta - bn2_mean * s2) + s2 * (conv1_b + w1 @ t1)  # [128]

    import ml_dtypes
    # w1t[p, b, m] = conv1_w[m, b*128+p]  (lhsT blocks for K=256 contraction)
    w1t = np.ascontiguousarray(
        conv1_w[:, :, 0, 0].T.reshape(2, 128, C_MID).transpose(1, 0, 2)
    ).astype(ml_dtypes.bfloat16)
    # w2t[k, dy*3+dx, m] = conv2_w[m, k, dy, dx]
    w2t = np.ascontiguousarray(
        conv2_w.transpose(1, 2, 3, 0).reshape(128, 9, C_OUT)
    ).astype(ml_dtypes.bfloat16)

    bnp = np.zeros((128, 8), np.float32)
    bnp[:, 0] = s1[0:128]
    bnp[:, 1] = s1[128:256]
    u1 = -t1 / s1
    bnp[:, 2] = u1[0:128]
    bnp[:, 3] = u1[128:256]
    bnp[:, 4] = s2
    bnp[:, 5] = t2
    bnp[0:C_OUT, 6] = conv2_b
    return w1t, w2t, bnp


def _get_runner():
    """Build (once) a jitted shard_map that runs the per-core NEFF on all 8
    cores. Mirrors bass2jax.run_bass_via_pjrt but caches the jit and keeps
    the output-backing zero buffer device-resident across calls."""
    if "runner" in _CACHE:
        return _CACHE["runner"]

    import jax
    from jax.sharding import Mesh, PartitionSpec
    from jax.experimental.shard_map import shard_map
    from concourse import bass2jax, mybir
    from concourse.bass2jax import _bass_exec_p, partition_id_tensor

    nc = _build_program()
    bass2jax.install_neuronx_cc_hook()

    partition_name = (nc.partition_id_tensor.name
                      if nc.partition_id_tensor else None)
    in_names = []
    out_names = []
    out_avals = []
    for alloc in nc.m.functions[0].allocations:
        if not isinstance(alloc, mybir.MemoryLocationSet):
            continue
        name = alloc.memorylocations[0].name
        if alloc.kind == "ExternalInput":
            if name != partition_name:
                in_names.append(name)
        elif alloc.kind == "ExternalOutput":
            out_names.append(name)
            out_avals.append(jax.core.ShapedArray(tuple(alloc.tensor_shape),
                                                  mybir.dt.np(alloc.dtype)))
    all_in_names = list(in_names) + list(out_names)
    if partition_name is not None:
        all_in_names.append(partition_name)

    def _body(*args):
        operands = list(args)
        if partition_name is not None:
            operands.append(partition_id_tensor())
        outs = _bass_exec_p.bind(
            *operands,
            out_avals=tuple(out_avals),
            in_names=tuple(all_in_names),
            out_names=tuple(out_names),
            lowering_input_output_aliases=(),
            sim_require_finite=True,
            sim_require_nnan=True,
            nc=nc,
        )
        return outs[0]

    devices = jax.devices()[:N_CORES]
    mesh = Mesh(np.asarray(devices), ("core",))
    nargs = len(in_names) + 1  # + output-backing buffer
    jitted = jax.jit(shard_map(_body, mesh=mesh,
                               in_specs=(PartitionSpec("core"),) * nargs,
                               out_specs=PartitionSpec("core"),
                               check_rep=False))
    zout = jax.device_put(
        np.zeros((N_CORES * N_PER_CORE, C_OUT, H, W), np.float32))
    _CACHE["runner"] = (jitted, list(in_names), zout)
    return _CACHE["runner"]


def kernel(x, bn1_gamma, bn1_beta, bn1_mean, bn1_var, conv1_w, conv1_b,
           bn2_gamma, bn2_beta, bn2_mean, bn2_var, conv2_w, conv2_b):
    x = np.ascontiguousarray(np.asarray(x, dtype=np.float32))
    w1t, w2t, bnp = _fold_params(
        np.asarray(conv1_w, np.float32), np.asarray(conv1_b, np.float32),
        np.asarray(conv2_w, np.float32), np.asarray(conv2_b, np.float32),
        np.asarray(bn1_gamma, np.float32), np.asarray(bn1_beta, np.float32),
        np.asarray(bn1_mean, np.float32), np.asarray(bn1_var, np.float32),
        np.asarray(bn2_gamma, np.float32), np.asarray(bn2_beta, np.float32),
        np.asarray(bn2_mean, np.float32), np.asarray(bn2_var, np.float32))

    jitted, in_names, zout = _get_runner()
    # global arrays: shard axis 0 across the 8 cores (params replicated)
    per_name = {
        "x": x,  # [32, ...] -> [4, ...] per core
        "w1t": np.concatenate([w1t] * N_CORES, axis=0),
        "w2t": np.concatenate([w2t] * N_CORES, axis=0),
        "bnp": np.concatenate([bnp] * N_CORES, axis=0),
    }
    args = [per_name[nm] for nm in in_names]
    comp = jitted(*args, zout)
    # unshard/gather: channels 0..255 of the concat are x verbatim and are
    # assembled here from the host-resident input; the device computes and
    # returns only the 32 conv output channels
    full = np.empty((N_CORES * N_PER_CORE, C_IN + C_OUT, H, W), np.float32)
    full[:, :C_IN] = x
    full[:, C_IN:] = np.asarray(comp)
    return full


# revision 23
# speedup vs baseline: 1.0243x; 1.0243x over previous
"""Fused BN+ReLU -> 1x1 conv -> BN+ReLU -> 3x3 conv -> concat kernel for TRN2.

Data-parallel over batch: 32 images are sharded 4-per-core across 8 NeuronCores.
BN params / conv weights are folded host-side and replicated.

Per-core structure (per image):
  - x image [256, 3136] is DMA'd to SBUF [128p, 2blk, 3136] in two row-chunks.
  - BN1+ReLU on the scalar engine (per-partition scale/bias), rounding to
    bf16 so the PE runs matmuls at full rate (4x faster than fp32) with
    weight loads hidden by the PE reorder window.
  - conv1 (1x1, 256->128): 2 accumulating matmuls per 8-row chunk into PSUM.
  - BN2+ReLU evicts PSUM into a zero-padded [128, 58, 58] image.
  - conv2 (3x3, 128->32): 9 shifted accumulating matmuls per 8-row chunk;
    DVE adds the conv2 bias into a staging tile; one DMA out per image.
The SP HWDGE stream carries only the x loads; params ride the ACT HWDGE and
conv2 outputs the Pool SWDGE so nothing head-of-line-blocks the loads. The
concat's verbatim x channels (0..255) are assembled on the host during the
gather/unshard step from the host-resident input, so device HBM traffic per
core is 12.85 MB in + 1.6 MB out.
"""

import numpy as np

EPS = 1e-5

N_CORES = 8
N_PER_CORE = 4          # 32 images / 8 cores
C_IN = 256
C_MID = 128
C_OUT = 32
H = W = 56
S = H * W               # 3136
PW = W + 2              # padded row width 58
PH = H + 2
ROWS = 8                # image rows per matmul chunk
NCHUNK = H // ROWS      # 7
CHUNK_N = ROWS * W      # 448 <= 512 (fp32 PSUM bank limit)
# x-load / BN1 pieces (in conv chunks): a small first piece so the first
# matmul starts ~5us in, then two larger ones
PIECES = [(0, 1), (1, 2), (2, 4), (4, 7)]  # chunk ranges per piece
PIECE_SL = [(lo * CHUNK_N, hi * CHUNK_N) for lo, hi in PIECES]

_CACHE = {}


def _build_program():
    if "nc" in _CACHE:
        return _CACHE["nc"]

    from contextlib import ExitStack

    import concourse.bacc as bacc
    import concourse.tile as tile
    from concourse import mybir

    f32 = mybir.dt.float32
    bf16 = mybir.dt.bfloat16  # matmul operands: full-rate PE + hidden LDWEIGHTS
    Relu = mybir.ActivationFunctionType.Relu

    nc = bacc.Bacc("TRN2", target_bir_lowering=False, debug=False,
                   num_devices=N_CORES)

    x_d = nc.declare_dram_parameter("x", [N_PER_CORE, C_IN, H, W], f32,
                                    isOutput=False)
    w1t_d = nc.declare_dram_parameter("w1t", [128, 2, C_MID], bf16,
                                      isOutput=False)
    w2t_d = nc.declare_dram_parameter("w2t", [128, 9, C_OUT], bf16,
                                      isOutput=False)
    bnp_d = nc.declare_dram_parameter("bnp", [128, 8], f32, isOutput=False)
    out_d = nc.declare_dram_parameter("out", [N_PER_CORE, C_OUT, H, W],
                                      f32, isOutput=True)

    with tile.TileContext(nc) as tc, ExitStack() as ctx:
        const = ctx.enter_context(tc.tile_pool(name="const", bufs=1))
        raw_p = ctx.enter_context(tc.tile_pool(name="raw", bufs=3))
        bn_p = ctx.enter_context(tc.tile_pool(name="bn", bufs=2))
        mid_p = ctx.enter_context(tc.tile_pool(name="mid", bufs=2))
        stage_p = ctx.enter_context(tc.tile_pool(name="stage", bufs=2))
        ps1_p = ctx.enter_context(tc.tile_pool(name="ps1", bufs=4, space="PSUM"))
        ps2_p = ctx.enter_context(tc.tile_pool(name="ps2", bufs=4, space="PSUM"))

        # params via the ACT-engine HWDGE: their DMA-resource requests land
        # ahead of the big x loads (tiny transfers), and they stay off the SP
        # HWDGE stream that feeds the x loads / copies
        bnp_s = const.tile([128, 8], f32)
        nc.scalar.dma_start(out=bnp_s[:], in_=bnp_d[:])
        w1t_s = const.tile([128, 2, C_MID], bf16)
        nc.scalar.dma_start(out=w1t_s[:], in_=w1t_d[:])
        w2t_s = const.tile([128, 9, C_OUT], bf16)
        nc.scalar.dma_start(out=w2t_s[:], in_=w2t_d[:])

        # tiny warm-up activation: hoists the ACT function-table load off the
        # first image's critical path
        warm = const.tile([128, 1], f32)
        nc.vector.memset(warm[:], 0.0)
        nc.scalar.activation(warm[:], warm[:], Relu)

        raws = {}

        def load_x(n):
            raws[n] = raw_p.tile([128, 2, S], f32, tag="raw", name=f"raw{n}")
            x_img = x_d[n].rearrange("(b p) h w -> p b (h w)", p=128)
            for lo, hi in PIECE_SL:
                nc.sync.dma_start(out=raws[n][:, :, lo:hi],
                                  in_=x_img[:, :, lo:hi])

        # SP HWDGE stream: all loads, back to back. raw bufs=3 means load 3
        # starts once image 0's BN1 has consumed its buffer.
        load_x(0)
        load_x(1)
        load_x(2)

        for n in range(N_PER_CORE):
            raw = raws.pop(n)
            if n + 3 < N_PER_CORE:
                load_x(n + 3)

            # BN1 + ReLU on DVE as z = s1*max(x, -t1/s1) (valid since s1>0);
            # the missing +t1 is folded into BN2's bias host-side (it passes
            # through the 1x1 conv as a constant). One tile per load piece so
            # conv1 chunk c only waits on its piece's two ops, and the DVE
            # (idle at start) takes BN1 off the ACT critical path.
            bns = []
            for pi, (lo, hi) in enumerate(PIECE_SL):
                bnt = bn_p.tile([128, 2, hi - lo], bf16, tag=f"bn{pi}",
                                name=f"bn{pi}_{n}")
                bns.append(bnt)
                for b in range(2):
                    nc.vector.tensor_scalar(
                        bnt[:, b], raw[:, b, lo:hi],
                        bnp_s[:, 2 + b:3 + b], bnp_s[:, b:b + 1],
                        mybir.AluOpType.max, mybir.AluOpType.mult)

            # zero-padded conv1 output image [128, 58, 58]
            mid = mid_p.tile([128, PH, PW], bf16)
            nc.gpsimd.memset(mid[:, 0, :], 0.0)
            nc.gpsimd.memset(mid[:, PH - 1, :], 0.0)
            nc.gpsimd.memset(mid[:, 1:PH - 1, 0:1], 0.0)
            nc.gpsimd.memset(mid[:, 1:PH - 1, PW - 1:PW], 0.0)

            # conv1 (1x1, 256->128) + BN2 + ReLU, 8 rows at a time
            for c in range(NCHUNK):
                pi = next(i for i, (lo, hi) in enumerate(PIECES)
                          if lo <= c < hi)
                bnc = bns[pi]
                off = (c - PIECES[pi][0]) * CHUNK_N
                ps1 = ps1_p.tile([128, ROWS, W], f32)
                for b in range(2):
                    nc.tensor.matmul(ps1[:], w1t_s[:, b],
                                     bnc[:, b, off:off + CHUNK_N],
                                     start=(b == 0), stop=(b == 1))
                nc.scalar.activation(mid[:, 1 + ROWS * c:1 + ROWS * (c + 1),
                                         1:1 + W],
                                     ps1[:], Relu,
                                     bias=bnp_s[:, 5:6], scale=bnp_s[:, 4:5])

            # conv2 (3x3, 128->32) as 9 shifted accumulating matmuls
            stage = stage_p.tile([C_OUT, S], f32)
            for c in range(NCHUNK):
                ps2 = ps2_p.tile([C_OUT, CHUNK_N], f32)
                for t in range(9):
                    dy, dx = divmod(t, 3)
                    rhs = mid[:, ROWS * c + dy:ROWS * c + dy + ROWS,
                              dx:dx + W]
                    nc.tensor.matmul(ps2[:], w2t_s[:, t], rhs,
                                     start=(t == 0), stop=(t == 8))
                # + conv2 bias, PSUM -> SBUF staging
                nc.vector.tensor_scalar_add(
                    stage[:, c * CHUNK_N:(c + 1) * CHUNK_N], ps2[:],
                    bnp_s[0:C_OUT, 6:7])

            # SWDGE on the (mostly idle) Pool engine so this DMA, which waits
            # on conv2, never head-of-line-blocks the SP x-load stream
            out_comp = out_d[n].rearrange("c h w -> c (h w)")
            nc.gpsimd.dma_start(out=out_comp, in_=stage[:])

    nc.compile()
    _CACHE["nc"] = nc
    return nc


def _fold_params(conv1_w, conv1_b, conv2_w, conv2_b,
                 bn1_gamma, bn1_beta, bn1_mean, bn1_var,
                 bn2_gamma, bn2_beta, bn2_mean, bn2_var):
    s1 = bn1_gamma / np.sqrt(bn1_var + EPS)                     # [256]
    t1 = bn1_beta - bn1_mean * s1                               # [256]
    s2 = bn2_gamma / np.sqrt(bn2_var + EPS)                     # [128]
    # conv1 bias, plus BN1's +t1 pushed through the 1x1 conv (see kernel)
    w1 = conv1_w[:, :, 0, 0]                                    # [128, 256]
    t2 = (bn2_beta - bn2_mean * s2) + s2 * (conv1_b + w1 @ t1)  # [128]

    import ml_dtypes
    # w1t[p, b, m] = conv1_w[m, b*128+p]  (lhsT blocks for K=256 contraction)
    w1t = np.ascontiguousarray(
        conv1_w[:, :, 0, 0].T.reshape(2, 128, C_MID).transpose(1, 0, 2)
    ).astype(ml_dtypes.bfloat16)
    # w2t[k, dy*3+dx, m] = conv2_w[m, k, dy, dx]
    w2t = np.ascontiguousarray(
        conv2_w.transpose(1, 2, 3, 0).reshape(128, 9, C_OUT)
    ).astype(ml_dtypes.bfloat16)

    bnp = np.zeros((128, 8), np.float32)
    bnp[:, 0] = s1[0:128]
    bnp[:, 1] = s1[128:256]
    u1 = -t1 / s1
    bnp[:, 2] = u1[0:128]
    bnp[:, 3] = u1[128:256]
    bnp[:, 4] = s2
    bnp[:, 5] = t2
    bnp[0:C_OUT, 6] = conv2_b
    return w1t, w2t, bnp


def _get_runner():
    """Build (once) a jitted shard_map that runs the per-core NEFF on all 8
    cores. Mirrors bass2jax.run_bass_via_pjrt but caches the jit and keeps
    the output-backing zero buffer device-resident across calls."""
    if "runner" in _CACHE:
        return _CACHE["runner"]

    import jax
    from jax.sharding import Mesh, PartitionSpec
    from jax.experimental.shard_map import shard_map
    from concourse import bass2jax, mybir
    from concourse.bass2jax import _bass_exec_p, partition_id_tensor

    nc = _build_program()
    bass2jax.install_neuronx_cc_hook()

    partition_name = (nc.partition_id_tensor.name
                      if nc.partition_id_tensor else None)
    in_names = []
    out_names = []
    out_avals = []
    for alloc in nc.m.functions[0].allocations:
        if not isinstance(alloc, mybir.MemoryLocationSet):
            continue
        name = alloc.memorylocations[0].name
        if alloc.kind == "ExternalInput":
            if name != partition_name:
                in_names.append(name)
        elif alloc.kind == "ExternalOutput":
            out_names.append(name)
            out_avals.append(jax.core.ShapedArray(tuple(alloc.tensor_shape),
                                                  mybir.dt.np(alloc.dtype)))
    all_in_names = list(in_names) + list(out_names)
    if partition_name is not None:
        all_in_names.append(partition_name)

    def _body(*args):
        operands = list(args)
        if partition_name is not None:
            operands.append(partition_id_tensor())
        outs = _bass_exec_p.bind(
            *operands,
            out_avals=tuple(out_avals),
            in_names=tuple(all_in_names),
            out_names=tuple(out_names),
            lowering_input_output_aliases=(),
            sim_require_finite=True,
            sim_require_nnan=True,
            nc=nc,
        )
        return outs[0]

    devices = jax.devices()[:N_CORES]
    mesh = Mesh(np.asarray(devices), ("core",))
    nargs = len(in_names) + 1  # + output-backing buffer
    jitted = jax.jit(shard_map(_body, mesh=mesh,
                               in_specs=(PartitionSpec("core"),) * nargs,
                               out_specs=PartitionSpec("core"),
                               check_rep=False))
    zout = jax.device_put(
        np.zeros((N_CORES * N_PER_CORE, C_OUT, H, W), np.float32))
    _CACHE["runner"] = (jitted, list(in_names), zout)
    return _CACHE["runner"]


def kernel(x, bn1_gamma, bn1_beta, bn1_mean, bn1_var, conv1_w, conv1_b,
           bn2_gamma, bn2_beta, bn2_mean, bn2_var, conv2_w, conv2_b):
    x = np.ascontiguousarray(np.asarray(x, dtype=np.float32))
    w1t, w2t, bnp = _fold_params(
        np.asarray(conv1_w, np.float32), np.asarray(conv1_b, np.float32),
        np.asarray(conv2_w, np.float32), np.asarray(conv2_b, np.float32),
        np.asarray(bn1_gamma, np.float32), np.asarray(bn1_beta, np.float32),
        np.asarray(bn1_mean, np.float32), np.asarray(bn1_var, np.float32),
        np.asarray(bn2_gamma, np.float32), np.asarray(bn2_beta, np.float32),
        np.asarray(bn2_mean, np.float32), np.asarray(bn2_var, np.float32))

    jitted, in_names, zout = _get_runner()
    # global arrays: shard axis 0 across the 8 cores (params replicated)
    per_name = {
        "x": x,  # [32, ...] -> [4, ...] per core
        "w1t": np.concatenate([w1t] * N_CORES, axis=0),
        "w2t": np.concatenate([w2t] * N_CORES, axis=0),
        "bnp": np.concatenate([bnp] * N_CORES, axis=0),
    }
    args = [per_name[nm] for nm in in_names]
    comp = jitted(*args, zout)
    # unshard/gather: channels 0..255 of the concat are x verbatim and are
    # assembled here from the host-resident input; the device computes and
    # returns only the 32 conv output channels
    full = np.empty((N_CORES * N_PER_CORE, C_IN + C_OUT, H, W), np.float32)
    full[:, :C_IN] = x
    full[:, C_IN:] = np.asarray(comp)
    return full


# revision 25
# speedup vs baseline: 1.0372x; 1.0126x over previous
"""Fused BN+ReLU -> 1x1 conv -> BN+ReLU -> 3x3 conv -> concat kernel for TRN2.

Data-parallel over batch: 32 images are sharded 4-per-core across 8 NeuronCores.
BN params / conv weights are folded host-side and replicated.

Per-core structure (per image):
  - x image [256, 3136] is DMA'd to SBUF [128p, 2blk, 3136] in two row-chunks.
  - BN1+ReLU on the scalar engine (per-partition scale/bias), rounding to
    bf16 so the PE runs matmuls at full rate (4x faster than fp32) with
    weight loads hidden by the PE reorder window.
  - conv1 (1x1, 256->128): 2 accumulating matmuls per 8-row chunk into PSUM.
  - BN2+ReLU evicts PSUM into a zero-padded [128, 58, 58] image.
  - conv2 (3x3, 128->32): 9 shifted accumulating matmuls per 8-row chunk;
    DVE adds the conv2 bias into a staging tile; one DMA out per image.
The SP HWDGE stream carries only the x loads; params ride the ACT HWDGE and
conv2 outputs the Pool SWDGE so nothing head-of-line-blocks the loads. The
concat's verbatim x channels (0..255) are assembled on the host during the
gather/unshard step from the host-resident input, so device HBM traffic per
core is 12.85 MB in + 1.6 MB out.
"""

import numpy as np

EPS = 1e-5

N_CORES = 8
N_PER_CORE = 4          # 32 images / 8 cores
C_IN = 256
C_MID = 128
C_OUT = 32
H = W = 56
S = H * W               # 3136
PW = W + 2              # padded row width 58
PH = H + 2
ROWS = 8                # image rows per matmul chunk
NCHUNK = H // ROWS      # 7
CHUNK_N = ROWS * W      # 448 <= 512 (fp32 PSUM bank limit)
# x-load / BN1 pieces (in conv chunks): a small first piece so the first
# matmul starts ~5us in, then two larger ones
PIECES = [(0, 1), (1, 2), (2, 4), (4, 7)]  # chunk ranges per piece
PIECE_SL = [(lo * CHUNK_N, hi * CHUNK_N) for lo, hi in PIECES]

_CACHE = {}


def _build_program():
    if "nc" in _CACHE:
        return _CACHE["nc"]

    from contextlib import ExitStack

    import concourse.bacc as bacc
    import concourse.tile as tile
    from concourse import mybir

    f32 = mybir.dt.float32
    bf16 = mybir.dt.bfloat16  # matmul operands: full-rate PE + hidden LDWEIGHTS
    Relu = mybir.ActivationFunctionType.Relu

    nc = bacc.Bacc("TRN2", target_bir_lowering=False, debug=False,
                   num_devices=N_CORES)

    x_d = nc.declare_dram_parameter("x", [N_PER_CORE, C_IN, H, W], f32,
                                    isOutput=False)
    w1t_d = nc.declare_dram_parameter("w1t", [128, 2, C_MID], bf16,
                                      isOutput=False)
    w2t_d = nc.declare_dram_parameter("w2t", [128, 9, C_OUT], bf16,
                                      isOutput=False)
    bnp_d = nc.declare_dram_parameter("bnp", [128, 8], f32, isOutput=False)
    out_d = nc.declare_dram_parameter("out", [N_PER_CORE, C_OUT, H, W],
                                      f32, isOutput=True)

    with tile.TileContext(nc) as tc, ExitStack() as ctx:
        const = ctx.enter_context(tc.tile_pool(name="const", bufs=1))
        raw_p = ctx.enter_context(tc.tile_pool(name="raw", bufs=3))
        bn_p = ctx.enter_context(tc.tile_pool(name="bn", bufs=2))
        mid_p = ctx.enter_context(tc.tile_pool(name="mid", bufs=2))
        stage_p = ctx.enter_context(tc.tile_pool(name="stage", bufs=2))
        ps1_p = ctx.enter_context(tc.tile_pool(name="ps1", bufs=4, space="PSUM"))
        ps2_p = ctx.enter_context(tc.tile_pool(name="ps2", bufs=4, space="PSUM"))

        # params via the ACT-engine HWDGE: their DMA-resource requests land
        # ahead of the big x loads (tiny transfers), and they stay off the SP
        # HWDGE stream that feeds the x loads / copies
        bnp_s = const.tile([128, 8], f32)
        nc.scalar.dma_start(out=bnp_s[:], in_=bnp_d[:])
        w1t_s = const.tile([128, 2, C_MID], bf16)
        nc.scalar.dma_start(out=w1t_s[:], in_=w1t_d[:])
        w2t_s = const.tile([128, 9, C_OUT], bf16)
        nc.scalar.dma_start(out=w2t_s[:], in_=w2t_d[:])

        # tiny warm-up activation: hoists the ACT function-table load off the
        # first image's critical path
        warm = const.tile([128, 1], f32)
        nc.vector.memset(warm[:], 0.0)
        nc.scalar.activation(warm[:], warm[:], Relu)

        raws = {}

        def load_x(n):
            raws[n] = raw_p.tile([128, 2, S], f32, tag="raw", name=f"raw{n}")
            x_img = x_d[n].rearrange("(b p) h w -> p b (h w)", p=128)
            for lo, hi in PIECE_SL:
                nc.sync.dma_start(out=raws[n][:, :, lo:hi],
                                  in_=x_img[:, :, lo:hi])

        # SP HWDGE stream: all loads, back to back. raw bufs=3 means load 3
        # starts once image 0's BN1 has consumed its buffer.
        load_x(0)
        load_x(1)
        load_x(2)

        for n in range(N_PER_CORE):
            raw = raws.pop(n)
            if n + 3 < N_PER_CORE:
                load_x(n + 3)

            # BN1 + ReLU on DVE as z = s1*max(x, -t1/s1) (valid since s1>0);
            # the missing +t1 is folded into BN2's bias host-side (it passes
            # through the 1x1 conv as a constant). One tile per load piece so
            # conv1 chunk c only waits on its piece's two ops, and the DVE
            # (idle at start) takes BN1 off the ACT critical path.
            bns = []
            for pi, (lo, hi) in enumerate(PIECE_SL):
                bnt = bn_p.tile([128, 2, hi - lo], bf16, tag=f"bn{pi}",
                                name=f"bn{pi}_{n}")
                bns.append(bnt)
                for b in range(2):
                    nc.vector.tensor_scalar(
                        bnt[:, b], raw[:, b, lo:hi],
                        bnp_s[:, 2 + b:3 + b], bnp_s[:, b:b + 1],
                        mybir.AluOpType.max, mybir.AluOpType.mult)

            # zero-padded conv1 output image [128, 58, 58]
            mid = mid_p.tile([128, PH, PW], bf16)
            nc.gpsimd.memset(mid[:, 0, :], 0.0)
            nc.gpsimd.memset(mid[:, PH - 1, :], 0.0)
            nc.gpsimd.memset(mid[:, 1:PH - 1, 0:1], 0.0)
            nc.gpsimd.memset(mid[:, 1:PH - 1, PW - 1:PW], 0.0)

            # conv1 (1x1, 256->128) + BN2 + ReLU, 8 rows at a time
            for c in range(NCHUNK):
                pi = next(i for i, (lo, hi) in enumerate(PIECES)
                          if lo <= c < hi)
                bnc = bns[pi]
                off = (c - PIECES[pi][0]) * CHUNK_N
                ps1 = ps1_p.tile([128, ROWS, W], f32)
                for b in range(2):
                    nc.tensor.matmul(ps1[:], w1t_s[:, b],
                                     bnc[:, b, off:off + CHUNK_N],
                                     start=(b == 0), stop=(b == 1))
                nc.scalar.activation(mid[:, 1 + ROWS * c:1 + ROWS * (c + 1),
                                         1:1 + W],
                                     ps1[:], Relu,
                                     bias=bnp_s[:, 5:6], scale=bnp_s[:, 4:5])

            # conv2 (3x3, 128->32) as 9 shifted accumulating matmuls
            stage = stage_p.tile([C_OUT, S], f32)
            out_comp = out_d[n].rearrange("c h w -> c (h w)")
            last_img = n == N_PER_CORE - 1
            for c in range(NCHUNK):
                ps2 = ps2_p.tile([C_OUT, CHUNK_N], f32)
                for t in range(9):
                    dy, dx = divmod(t, 3)
                    rhs = mid[:, ROWS * c + dy:ROWS * c + dy + ROWS,
                              dx:dx + W]
                    nc.tensor.matmul(ps2[:], w2t_s[:, t], rhs,
                                     start=(t == 0), stop=(t == 8))
                # + conv2 bias, PSUM -> SBUF staging
                nc.vector.tensor_scalar_add(
                    stage[:, c * CHUNK_N:(c + 1) * CHUNK_N], ps2[:],
                    bnp_s[0:C_OUT, 6:7])
                # last image: ship chunks 0..5 early so only the final 57KB
                # chunk's DMA sits on the critical tail
                if last_img and c == NCHUNK - 2:
                    nc.gpsimd.dma_start(
                        out=out_comp[:, 0:(NCHUNK - 1) * CHUNK_N],
                        in_=stage[:, 0:(NCHUNK - 1) * CHUNK_N])

            # SWDGE on the (mostly idle) Pool engine so this DMA, which waits
            # on conv2, never head-of-line-blocks the SP x-load stream
            if last_img:
                nc.gpsimd.dma_start(
                    out=out_comp[:, (NCHUNK - 1) * CHUNK_N:S],
                    in_=stage[:, (NCHUNK - 1) * CHUNK_N:S])
            else:
                nc.gpsimd.dma_start(out=out_comp, in_=stage[:])

    nc.compile()
    _CACHE["nc"] = nc
    return nc


def _fold_params(conv1_w, conv1_b, conv2_w, conv2_b,
                 bn1_gamma, bn1_beta, bn1_mean, bn1_var,
                 bn2_gamma, bn2_beta, bn2_mean, bn2_var):
    s1 = bn1_gamma / np.sqrt(bn1_var + EPS)                     # [256]
    t1 = bn1_beta - bn1_mean * s1                               # [256]
    s2 = bn2_gamma / np.sqrt(bn2_var + EPS)                     # [128]
    # conv1 bias, plus BN1's +t1 pushed through the 1x1 conv (see kernel)
    w1 = conv1_w[:, :, 0, 0]                                    # [128, 256]
    t2 = (bn2_beta - bn2_mean * s2) + s2 * (conv1_b + w1 @ t1)  # [128]

    import ml_dtypes
    # w1t[p, b, m] = conv1_w[m, b*128+p]  (lhsT blocks for K=256 contraction)
    w1t = np.ascontiguousarray(
        conv1_w[:, :, 0, 0].T.reshape(2, 128, C_MID).transpose(1, 0, 2)
    ).astype(ml_dtypes.bfloat16)
    # w2t[k, dy*3+dx, m] = conv2_w[m, k, dy, dx]
    w2t = np.ascontiguousarray(
        conv2_w.transpose(1, 2, 3, 0).reshape(128, 9, C_OUT)
    ).astype(ml_dtypes.bfloat16)

    bnp = np.zeros((128, 8), np.float32)
    bnp[:, 0] = s1[0:128]
    bnp[:, 1] = s1[128:256]
    u1 = -t1 / s1
    bnp[:, 2] = u1[0:128]
    bnp[:, 3] = u1[128:256]
    bnp[:, 4] = s2
    bnp[:, 5] = t2
    bnp[0:C_OUT, 6] = conv2_b
    return w1t, w2t, bnp


def _get_runner():
    """Build (once) a jitted shard_map that runs the per-core NEFF on all 8
    cores. Mirrors bass2jax.run_bass_via_pjrt but caches the jit and keeps
    the output-backing zero buffer device-resident across calls."""
    if "runner" in _CACHE:
        return _CACHE["runner"]

    import jax
    from jax.sharding import Mesh, PartitionSpec
    from jax.experimental.shard_map import shard_map
    from concourse import bass2jax, mybir
    from concourse.bass2jax import _bass_exec_p, partition_id_tensor

    nc = _build_program()
    bass2jax.install_neuronx_cc_hook()

    partition_name = (nc.partition_id_tensor.name
                      if nc.partition_id_tensor else None)
    in_names = []
    out_names = []
    out_avals = []
    for alloc in nc.m.functions[0].allocations:
        if not isinstance(alloc, mybir.MemoryLocationSet):
            continue
        name = alloc.memorylocations[0].name
        if alloc.kind == "ExternalInput":
            if name != partition_name:
                in_names.append(name)
        elif alloc.kind == "ExternalOutput":
            out_names.append(name)
            out_avals.append(jax.core.ShapedArray(tuple(alloc.tensor_shape),
                                                  mybir.dt.np(alloc.dtype)))
    all_in_names = list(in_names) + list(out_names)
    if partition_name is not None:
        all_in_names.append(partition_name)

    def _body(*args):
        operands = list(args)
        if partition_name is not None:
            operands.append(partition_id_tensor())
        outs = _bass_exec_p.bind(
            *operands,
            out_avals=tuple(out_avals),
            in_names=tuple(all_in_names),
            out_names=tuple(out_names),
            lowering_input_output_aliases=(),
            sim_require_finite=True,
            sim_require_nnan=True,
            nc=nc,
        )
        return outs[0]

    devices = jax.devices()[:N_CORES]
    mesh = Mesh(np.asarray(devices), ("core",))
    nargs = len(in_names) + 1  # + output-backing buffer
    jitted = jax.jit(shard_map(_body, mesh=mesh,
                               in_specs=(PartitionSpec("core"),) * nargs,
                               out_specs=PartitionSpec("core"),
                               check_rep=False))
    zout = jax.device_put(
        np.zeros((N_CORES * N_PER_CORE, C_OUT, H, W), np.float32))
    _CACHE["runner"] = (jitted, list(in_names), zout)
    return _CACHE["runner"]


def kernel(x, bn1_gamma, bn1_beta, bn1_mean, bn1_var, conv1_w, conv1_b,
           bn2_gamma, bn2_beta, bn2_mean, bn2_var, conv2_w, conv2_b):
    x = np.ascontiguousarray(np.asarray(x, dtype=np.float32))
    w1t, w2t, bnp = _fold_params(
        np.asarray(conv1_w, np.float32), np.asarray(conv1_b, np.float32),
        np.asarray(conv2_w, np.float32), np.asarray(conv2_b, np.float32),
        np.asarray(bn1_gamma, np.float32), np.asarray(bn1_beta, np.float32),
        np.asarray(bn1_mean, np.float32), np.asarray(bn1_var, np.float32),
        np.asarray(bn2_gamma, np.float32), np.asarray(bn2_beta, np.float32),
        np.asarray(bn2_mean, np.float32), np.asarray(bn2_var, np.float32))

    jitted, in_names, zout = _get_runner()
    # global arrays: shard axis 0 across the 8 cores (params replicated)
    per_name = {
        "x": x,  # [32, ...] -> [4, ...] per core
        "w1t": np.concatenate([w1t] * N_CORES, axis=0),
        "w2t": np.concatenate([w2t] * N_CORES, axis=0),
        "bnp": np.concatenate([bnp] * N_CORES, axis=0),
    }
    args = [per_name[nm] for nm in in_names]
    comp = jitted(*args, zout)
    # unshard/gather: channels 0..255 of the concat are x verbatim and are
    # assembled here from the host-resident input; the device computes and
    # returns only the 32 conv output channels
    full = np.empty((N_CORES * N_PER_CORE, C_IN + C_OUT, H, W), np.float32)
    full[:, :C_IN] = x
    full[:, C_IN:] = np.asarray(comp)
    return full


# revision 28
# speedup vs baseline: 1.0399x; 1.0026x over previous
"""Fused BN+ReLU -> 1x1 conv -> BN+ReLU -> 3x3 conv -> concat kernel for TRN2.

Data-parallel over batch: 32 images are sharded 4-per-core across 8 NeuronCores.
BN params / conv weights are folded host-side and replicated.

Per-core structure (per image):
  - x image [256, 3136] is DMA'd to SBUF [128p, 2blk, 3136] in two row-chunks.
  - BN1+ReLU on the scalar engine (per-partition scale/bias), rounding to
    bf16 so the PE runs matmuls at full rate (4x faster than fp32) with
    weight loads hidden by the PE reorder window.
  - conv1 (1x1, 256->128): 2 accumulating matmuls per 8-row chunk into PSUM.
  - BN2+ReLU evicts PSUM into a zero-padded [128, 58, 58] image.
  - conv2 (3x3, 128->32): 9 shifted accumulating matmuls per 8-row chunk;
    DVE adds the conv2 bias into a staging tile; one DMA out per image.
The SP HWDGE stream carries only the x loads; params ride the ACT HWDGE and
conv2 outputs the Pool SWDGE so nothing head-of-line-blocks the loads. The
concat's verbatim x channels (0..255) are assembled on the host during the
gather/unshard step from the host-resident input, so device HBM traffic per
core is 12.85 MB in + 1.6 MB out.
"""

import numpy as np

EPS = 1e-5

N_CORES = 8
N_PER_CORE = 4          # 32 images / 8 cores
C_IN = 256
C_MID = 128
C_OUT = 32
H = W = 56
S = H * W               # 3136
PW = W + 2              # padded row width 58
PH = H + 2
ROWS = 8                # image rows per matmul chunk
NCHUNK = H // ROWS      # 7
CHUNK_N = ROWS * W      # 448 <= 512 (fp32 PSUM bank limit)
# x-load / BN1 pieces (in conv chunks): a small first piece so the first
# matmul starts ~5us in, then two larger ones
PIECES = [(0, 1), (1, 2), (2, 4), (4, 7)]  # chunk ranges per piece
PIECE_SL = [(lo * CHUNK_N, hi * CHUNK_N) for lo, hi in PIECES]

_CACHE = {}


def _build_program():
    if "nc" in _CACHE:
        return _CACHE["nc"]

    from contextlib import ExitStack

    import concourse.bacc as bacc
    import concourse.tile as tile
    from concourse import mybir

    f32 = mybir.dt.float32
    bf16 = mybir.dt.bfloat16  # matmul operands: full-rate PE + hidden LDWEIGHTS
    Relu = mybir.ActivationFunctionType.Relu

    nc = bacc.Bacc("TRN2", target_bir_lowering=False, debug=False,
                   num_devices=N_CORES)

    x_d = nc.declare_dram_parameter("x", [N_PER_CORE, C_IN, H, W], f32,
                                    isOutput=False)
    w1t_d = nc.declare_dram_parameter("w1t", [128, 2, C_MID], bf16,
                                      isOutput=False)
    w2t_d = nc.declare_dram_parameter("w2t", [128, 9, C_OUT], bf16,
                                      isOutput=False)
    bnp_d = nc.declare_dram_parameter("bnp", [128, 8], f32, isOutput=False)
    out_d = nc.declare_dram_parameter("out", [N_PER_CORE, C_OUT, H, W],
                                      f32, isOutput=True)

    with tile.TileContext(nc) as tc, ExitStack() as ctx:
        const = ctx.enter_context(tc.tile_pool(name="const", bufs=1))
        raw_p = ctx.enter_context(tc.tile_pool(name="raw", bufs=3))
        bn_p = ctx.enter_context(tc.tile_pool(name="bn", bufs=2))
        mid_p = ctx.enter_context(tc.tile_pool(name="mid", bufs=2))
        stage_p = ctx.enter_context(tc.tile_pool(name="stage", bufs=2))
        ps1_p = ctx.enter_context(tc.tile_pool(name="ps1", bufs=4, space="PSUM"))
        ps2_p = ctx.enter_context(tc.tile_pool(name="ps2", bufs=4, space="PSUM"))

        # params via the ACT-engine HWDGE: their DMA-resource requests land
        # ahead of the big x loads (tiny transfers), and they stay off the SP
        # HWDGE stream that feeds the x loads / copies
        bnp_s = const.tile([128, 8], f32)
        nc.scalar.dma_start(out=bnp_s[:], in_=bnp_d[:])
        w1t_s = const.tile([128, 2, C_MID], bf16)
        nc.scalar.dma_start(out=w1t_s[:], in_=w1t_d[:])
        w2t_s = const.tile([128, 9, C_OUT], bf16)
        nc.scalar.dma_start(out=w2t_s[:], in_=w2t_d[:])

        # tiny warm-up activation: hoists the ACT function-table load off the
        # first image's critical path
        warm = const.tile([128, 1], f32)
        nc.vector.memset(warm[:], 0.0)
        nc.scalar.activation(warm[:], warm[:], Relu)

        raws = {}

        def load_x(n):
            raws[n] = raw_p.tile([128, 2, S], f32, tag="raw", name=f"raw{n}")
            x_img = x_d[n].rearrange("(b p) h w -> p b (h w)", p=128)
            for lo, hi in PIECE_SL:
                nc.sync.dma_start(out=raws[n][:, :, lo:hi],
                                  in_=x_img[:, :, lo:hi])

        # SP HWDGE stream: all loads, back to back. raw bufs=3 means load 3
        # starts once image 0's BN1 has consumed its buffer.
        load_x(0)
        load_x(1)
        load_x(2)

        for n in range(N_PER_CORE):
            raw = raws.pop(n)
            if n + 3 < N_PER_CORE:
                load_x(n + 3)

            # BN1 + ReLU, the two channel blocks on different engines in
            # parallel: block 0 on DVE as z = s1*max(x, -t1/s1) (s1>0; its
            # +t1 is folded into BN2's bias host-side, passing through the
            # 1x1 conv as a constant), block 1 on ACT as relu(s1*x + t1).
            # One tile per load piece so conv1 chunk c only waits on its
            # piece's two ops.
            bns = []
            for pi, (lo, hi) in enumerate(PIECE_SL):
                bnt = bn_p.tile([128, 2, hi - lo], bf16, tag=f"bn{pi}",
                                name=f"bn{pi}_{n}")
                bns.append(bnt)
                nc.vector.tensor_scalar(
                    bnt[:, 0], raw[:, 0, lo:hi],
                    bnp_s[:, 2:3], bnp_s[:, 0:1],
                    mybir.AluOpType.max, mybir.AluOpType.mult)
                nc.scalar.activation(bnt[:, 1], raw[:, 1, lo:hi], Relu,
                                     bias=bnp_s[:, 3:4],
                                     scale=bnp_s[:, 1:2])

            # zero-padded conv1 output image [128, 58, 58]
            mid = mid_p.tile([128, PH, PW], bf16)
            nc.gpsimd.memset(mid[:, 0, :], 0.0)
            nc.gpsimd.memset(mid[:, PH - 1, :], 0.0)
            nc.gpsimd.memset(mid[:, 1:PH - 1, 0:1], 0.0)
            nc.gpsimd.memset(mid[:, 1:PH - 1, PW - 1:PW], 0.0)

            # conv1 (1x1, 256->128) + BN2 + ReLU, 8 rows at a time
            for c in range(NCHUNK):
                pi = next(i for i, (lo, hi) in enumerate(PIECES)
                          if lo <= c < hi)
                bnc = bns[pi]
                off = (c - PIECES[pi][0]) * CHUNK_N
                ps1 = ps1_p.tile([128, ROWS, W], f32)
                for b in range(2):
                    nc.tensor.matmul(ps1[:], w1t_s[:, b],
                                     bnc[:, b, off:off + CHUNK_N],
                                     start=(b == 0), stop=(b == 1))
                nc.scalar.activation(mid[:, 1 + ROWS * c:1 + ROWS * (c + 1),
                                         1:1 + W],
                                     ps1[:], Relu,
                                     bias=bnp_s[:, 5:6], scale=bnp_s[:, 4:5])

            # conv2 (3x3, 128->32) as 9 shifted accumulating matmuls
            stage = stage_p.tile([C_OUT, S], f32)
            out_comp = out_d[n].rearrange("c h w -> c (h w)")
            last_img = n == N_PER_CORE - 1
            for c in range(NCHUNK):
                ps2 = ps2_p.tile([C_OUT, CHUNK_N], f32)
                for t in range(9):
                    dy, dx = divmod(t, 3)
                    rhs = mid[:, ROWS * c + dy:ROWS * c + dy + ROWS,
                              dx:dx + W]
                    nc.tensor.matmul(ps2[:], w2t_s[:, t], rhs,
                                     start=(t == 0), stop=(t == 8))
                # + conv2 bias, PSUM -> SBUF staging
                nc.vector.tensor_scalar_add(
                    stage[:, c * CHUNK_N:(c + 1) * CHUNK_N], ps2[:],
                    bnp_s[0:C_OUT, 6:7])
                # last image: ship chunks 0..5 early so only the final 57KB
                # chunk's DMA sits on the critical tail
                if last_img and c == NCHUNK - 2:
                    nc.gpsimd.dma_start(
                        out=out_comp[:, 0:(NCHUNK - 1) * CHUNK_N],
                        in_=stage[:, 0:(NCHUNK - 1) * CHUNK_N])

            # SWDGE on the (mostly idle) Pool engine so this DMA, which waits
            # on conv2, never head-of-line-blocks the SP x-load stream
            if last_img:
                nc.gpsimd.dma_start(
                    out=out_comp[:, (NCHUNK - 1) * CHUNK_N:S],
                    in_=stage[:, (NCHUNK - 1) * CHUNK_N:S])
            else:
                nc.gpsimd.dma_start(out=out_comp, in_=stage[:])

    nc.compile()
    _CACHE["nc"] = nc
    return nc


def _fold_params(conv1_w, conv1_b, conv2_w, conv2_b,
                 bn1_gamma, bn1_beta, bn1_mean, bn1_var,
                 bn2_gamma, bn2_beta, bn2_mean, bn2_var):
    s1 = bn1_gamma / np.sqrt(bn1_var + EPS)                     # [256]
    t1 = bn1_beta - bn1_mean * s1                               # [256]
    s2 = bn2_gamma / np.sqrt(bn2_var + EPS)                     # [128]
    # conv1 bias, plus block 0's BN1 +t1 pushed through the 1x1 conv
    # (block 1 applies its t1 directly on the ACT engine; see kernel)
    w1 = conv1_w[:, :, 0, 0]                                    # [128, 256]
    t2 = (bn2_beta - bn2_mean * s2) + s2 * (
        conv1_b + w1[:, 0:128] @ t1[0:128])                     # [128]

    import ml_dtypes
    # w1t[p, b, m] = conv1_w[m, b*128+p]  (lhsT blocks for K=256 contraction)
    w1t = np.ascontiguousarray(
        conv1_w[:, :, 0, 0].T.reshape(2, 128, C_MID).transpose(1, 0, 2)
    ).astype(ml_dtypes.bfloat16)
    # w2t[k, dy*3+dx, m] = conv2_w[m, k, dy, dx]
    w2t = np.ascontiguousarray(
        conv2_w.transpose(1, 2, 3, 0).reshape(128, 9, C_OUT)
    ).astype(ml_dtypes.bfloat16)

    bnp = np.zeros((128, 8), np.float32)
    bnp[:, 0] = s1[0:128]
    bnp[:, 1] = s1[128:256]
    bnp[:, 2] = -t1[0:128] / s1[0:128]   # u for the DVE max-mult form
    bnp[:, 3] = t1[128:256]              # plain bias for the ACT form
    bnp[:, 4] = s2
    bnp[:, 5] = t2
    bnp[0:C_OUT, 6] = conv2_b
    return w1t, w2t, bnp


def _get_runner():
    """Build (once) a jitted shard_map that runs the per-core NEFF on all 8
    cores. Mirrors bass2jax.run_bass_via_pjrt but caches the jit and keeps
    the output-backing zero buffer device-resident across calls."""
    if "runner" in _CACHE:
        return _CACHE["runner"]

    import jax
    from jax.sharding import Mesh, PartitionSpec
    from jax.experimental.shard_map import shard_map
    from concourse import bass2jax, mybir
    from concourse.bass2jax import _bass_exec_p, partition_id_tensor

    nc = _build_program()
    bass2jax.install_neuronx_cc_hook()

    partition_name = (nc.partition_id_tensor.name
                      if nc.partition_id_tensor else None)
    in_names = []
    out_names = []
    out_avals = []
    for alloc in nc.m.functions[0].allocations:
        if not isinstance(alloc, mybir.MemoryLocationSet):
            continue
        name = alloc.memorylocations[0].name
        if alloc.kind == "ExternalInput":
            if name != partition_name:
                in_names.append(name)
        elif alloc.kind == "ExternalOutput":
            out_names.append(name)
            out_avals.append(jax.core.ShapedArray(tuple(alloc.tensor_shape),
                                                  mybir.dt.np(alloc.dtype)))
    all_in_names = list(in_names) + list(out_names)
    if partition_name is not None:
        all_in_names.append(partition_name)

    def _body(*args):
        operands = list(args)
        if partition_name is not None:
            operands.append(partition_id_tensor())
        outs = _bass_exec_p.bind(
            *operands,
            out_avals=tuple(out_avals),
            in_names=tuple(all_in_names),
            out_names=tuple(out_names),
            lowering_input_output_aliases=(),
            sim_require_finite=True,
            sim_require_nnan=True,
            nc=nc,
        )
        return outs[0]

    devices = jax.devices()[:N_CORES]
    mesh = Mesh(np.asarray(devices), ("core",))
    nargs = len(in_names) + 1  # + output-backing buffer
    jitted = jax.jit(shard_map(_body, mesh=mesh,
                               in_specs=(PartitionSpec("core"),) * nargs,
                               out_specs=PartitionSpec("core"),
                               check_rep=False))
    zout = jax.device_put(
        np.zeros((N_CORES * N_PER_CORE, C_OUT, H, W), np.float32))
    _CACHE["runner"] = (jitted, list(in_names), zout)
    return _CACHE["runner"]


def kernel(x, bn1_gamma, bn1_beta, bn1_mean, bn1_var, conv1_w, conv1_b,
           bn2_gamma, bn2_beta, bn2_mean, bn2_var, conv2_w, conv2_b):
    x = np.ascontiguousarray(np.asarray(x, dtype=np.float32))
    w1t, w2t, bnp = _fold_params(
        np.asarray(conv1_w, np.float32), np.asarray(conv1_b, np.float32),
        np.asarray(conv2_w, np.float32), np.asarray(conv2_b, np.float32),
        np.asarray(bn1_gamma, np.float32), np.asarray(bn1_beta, np.float32),
        np.asarray(bn1_mean, np.float32), np.asarray(bn1_var, np.float32),
        np.asarray(bn2_gamma, np.float32), np.asarray(bn2_beta, np.float32),
        np.asarray(bn2_mean, np.float32), np.asarray(bn2_var, np.float32))

    jitted, in_names, zout = _get_runner()
    # global arrays: shard axis 0 across the 8 cores (params replicated)
    per_name = {
        "x": x,  # [32, ...] -> [4, ...] per core
        "w1t": np.concatenate([w1t] * N_CORES, axis=0),
        "w2t": np.concatenate([w2t] * N_CORES, axis=0),
        "bnp": np.concatenate([bnp] * N_CORES, axis=0),
    }
    args = [per_name[nm] for nm in in_names]
    comp = jitted(*args, zout)
    # unshard/gather: channels 0..255 of the concat are x verbatim and are
    # assembled here from the host-resident input; the device computes and
    # returns only the 32 conv output channels
    full = np.empty((N_CORES * N_PER_CORE, C_IN + C_OUT, H, W), np.float32)
    full[:, :C_IN] = x
    full[:, C_IN:] = np.asarray(comp)
    return full


# revision 29
# speedup vs baseline: 1.0464x; 1.0063x over previous
"""Fused BN+ReLU -> 1x1 conv -> BN+ReLU -> 3x3 conv -> concat kernel for TRN2.

Data-parallel over batch: 32 images are sharded 4-per-core across 8 NeuronCores.
BN params / conv weights are folded host-side and replicated.

Per-core structure (per image):
  - x image [256, 3136] is DMA'd to SBUF [128p, 2blk, 3136] in two row-chunks.
  - BN1+ReLU on the scalar engine (per-partition scale/bias), rounding to
    bf16 so the PE runs matmuls at full rate (4x faster than fp32) with
    weight loads hidden by the PE reorder window.
  - conv1 (1x1, 256->128): 2 accumulating matmuls per 8-row chunk into PSUM.
  - BN2+ReLU evicts PSUM into a zero-padded [128, 58, 58] image.
  - conv2 (3x3, 128->32): 9 shifted accumulating matmuls per 8-row chunk;
    DVE adds the conv2 bias into a staging tile; one DMA out per image.
The SP HWDGE stream carries only the x loads; params ride the ACT HWDGE and
conv2 outputs the Pool SWDGE so nothing head-of-line-blocks the loads. The
concat's verbatim x channels (0..255) are assembled on the host during the
gather/unshard step from the host-resident input, so device HBM traffic per
core is 12.85 MB in + 1.6 MB out.
"""

import numpy as np

EPS = 1e-5

N_CORES = 8
N_PER_CORE = 4          # 32 images / 8 cores
C_IN = 256
C_MID = 128
C_OUT = 32
H = W = 56
S = H * W               # 3136
PW = W + 2              # padded row width 58
PH = H + 2
ROWS = 8                # image rows per matmul chunk
NCHUNK = H // ROWS      # 7
CHUNK_N = ROWS * W      # 448 <= 512 (fp32 PSUM bank limit)
# x-load / BN1 pieces (column ranges): a tiny first piece so the first
# matmul (a 4-row half of chunk 0) issues as early as possible
PIECE_SL = [(0, 224), (224, 448), (448, 896), (896, 1792), (1792, S)]

_CACHE = {}


def _build_program():
    if "nc" in _CACHE:
        return _CACHE["nc"]

    from contextlib import ExitStack

    import concourse.bacc as bacc
    import concourse.tile as tile
    from concourse import mybir

    f32 = mybir.dt.float32
    bf16 = mybir.dt.bfloat16  # matmul operands: full-rate PE + hidden LDWEIGHTS
    Relu = mybir.ActivationFunctionType.Relu

    nc = bacc.Bacc("TRN2", target_bir_lowering=False, debug=False,
                   num_devices=N_CORES)

    x_d = nc.declare_dram_parameter("x", [N_PER_CORE, C_IN, H, W], f32,
                                    isOutput=False)
    w1t_d = nc.declare_dram_parameter("w1t", [128, 2, C_MID], bf16,
                                      isOutput=False)
    w2t_d = nc.declare_dram_parameter("w2t", [128, 9, C_OUT], bf16,
                                      isOutput=False)
    bnp_d = nc.declare_dram_parameter("bnp", [128, 8], f32, isOutput=False)
    out_d = nc.declare_dram_parameter("out", [N_PER_CORE, C_OUT, H, W],
                                      f32, isOutput=True)

    with tile.TileContext(nc) as tc, ExitStack() as ctx:
        const = ctx.enter_context(tc.tile_pool(name="const", bufs=1))
        raw_p = ctx.enter_context(tc.tile_pool(name="raw", bufs=3))
        bn_p = ctx.enter_context(tc.tile_pool(name="bn", bufs=2))
        mid_p = ctx.enter_context(tc.tile_pool(name="mid", bufs=2))
        stage_p = ctx.enter_context(tc.tile_pool(name="stage", bufs=2))
        ps1_p = ctx.enter_context(tc.tile_pool(name="ps1", bufs=4, space="PSUM"))
        ps2_p = ctx.enter_context(tc.tile_pool(name="ps2", bufs=4, space="PSUM"))

        # params via the ACT-engine HWDGE: their DMA-resource requests land
        # ahead of the big x loads (tiny transfers), and they stay off the SP
        # HWDGE stream that feeds the x loads / copies
        bnp_s = const.tile([128, 8], f32)
        nc.scalar.dma_start(out=bnp_s[:], in_=bnp_d[:])
        w1t_s = const.tile([128, 2, C_MID], bf16)
        nc.scalar.dma_start(out=w1t_s[:], in_=w1t_d[:])
        w2t_s = const.tile([128, 9, C_OUT], bf16)
        nc.scalar.dma_start(out=w2t_s[:], in_=w2t_d[:])

        # tiny warm-up activation: hoists the ACT function-table load off the
        # first image's critical path
        warm = const.tile([128, 1], f32)
        nc.vector.memset(warm[:], 0.0)
        nc.scalar.activation(warm[:], warm[:], Relu)

        raws = {}

        def load_x(n):
            raws[n] = raw_p.tile([128, 2, S], f32, tag="raw", name=f"raw{n}")
            x_img = x_d[n].rearrange("(b p) h w -> p b (h w)", p=128)
            for lo, hi in PIECE_SL:
                nc.sync.dma_start(out=raws[n][:, :, lo:hi],
                                  in_=x_img[:, :, lo:hi])

        # SP HWDGE stream: all loads, back to back. raw bufs=3 means load 3
        # starts once image 0's BN1 has consumed its buffer.
        load_x(0)
        load_x(1)
        load_x(2)

        for n in range(N_PER_CORE):
            raw = raws.pop(n)
            if n + 3 < N_PER_CORE:
                load_x(n + 3)

            # BN1 + ReLU, the two channel blocks on different engines in
            # parallel: block 0 on DVE as z = s1*max(x, -t1/s1) (s1>0; its
            # +t1 is folded into BN2's bias host-side, passing through the
            # 1x1 conv as a constant), block 1 on ACT as relu(s1*x + t1).
            # One tile per load piece so conv1 chunk c only waits on its
            # piece's two ops.
            bns = []
            for pi, (lo, hi) in enumerate(PIECE_SL):
                bnt = bn_p.tile([128, 2, hi - lo], bf16, tag=f"bn{pi}",
                                name=f"bn{pi}_{n}")
                bns.append(bnt)
                nc.vector.tensor_scalar(
                    bnt[:, 0], raw[:, 0, lo:hi],
                    bnp_s[:, 2:3], bnp_s[:, 0:1],
                    mybir.AluOpType.max, mybir.AluOpType.mult)
                nc.scalar.activation(bnt[:, 1], raw[:, 1, lo:hi], Relu,
                                     bias=bnp_s[:, 3:4],
                                     scale=bnp_s[:, 1:2])

            # zero-padded conv1 output image [128, 58, 58]
            mid = mid_p.tile([128, PH, PW], bf16)
            nc.gpsimd.memset(mid[:, 0, :], 0.0)
            nc.gpsimd.memset(mid[:, PH - 1, :], 0.0)
            nc.gpsimd.memset(mid[:, 1:PH - 1, 0:1], 0.0)
            nc.gpsimd.memset(mid[:, 1:PH - 1, PW - 1:PW], 0.0)

            # conv1 (1x1, 256->128) + BN2 + ReLU, 8 rows at a time;
            # chunk 0 runs as two 4-row halves so its first matmul only
            # needs the tiny first load piece
            for c in range(NCHUNK):
                sub = 2 if c == 0 else 1
                rs = ROWS // sub
                for sh in range(sub):
                    col0 = c * CHUNK_N + sh * rs * W
                    ncols = rs * W
                    pi = next(i for i, (lo, hi) in enumerate(PIECE_SL)
                              if lo <= col0 and col0 + ncols <= hi)
                    bnc = bns[pi]
                    off = col0 - PIECE_SL[pi][0]
                    ps1 = ps1_p.tile([128, rs, W], f32, tag="ps1",
                                     name=f"ps1_{n}_{c}_{sh}")
                    for b in range(2):
                        nc.tensor.matmul(ps1[:], w1t_s[:, b],
                                         bnc[:, b, off:off + ncols],
                                         start=(b == 0), stop=(b == 1))
                    row0 = 1 + ROWS * c + rs * sh
                    nc.scalar.activation(mid[:, row0:row0 + rs, 1:1 + W],
                                         ps1[:], Relu,
                                         bias=bnp_s[:, 5:6],
                                         scale=bnp_s[:, 4:5])

            # conv2 (3x3, 128->32) as 9 shifted accumulating matmuls
            stage = stage_p.tile([C_OUT, S], f32)
            out_comp = out_d[n].rearrange("c h w -> c (h w)")
            last_img = n == N_PER_CORE - 1
            for c in range(NCHUNK):
                ps2 = ps2_p.tile([C_OUT, CHUNK_N], f32)
                for t in range(9):
                    dy, dx = divmod(t, 3)
                    rhs = mid[:, ROWS * c + dy:ROWS * c + dy + ROWS,
                              dx:dx + W]
                    nc.tensor.matmul(ps2[:], w2t_s[:, t], rhs,
                                     start=(t == 0), stop=(t == 8))
                # + conv2 bias, PSUM -> SBUF staging
                nc.vector.tensor_scalar_add(
                    stage[:, c * CHUNK_N:(c + 1) * CHUNK_N], ps2[:],
                    bnp_s[0:C_OUT, 6:7])
                # last image: ship chunks 0..5 early so only the final 57KB
                # chunk's DMA sits on the critical tail
                if last_img and c == NCHUNK - 2:
                    nc.gpsimd.dma_start(
                        out=out_comp[:, 0:(NCHUNK - 1) * CHUNK_N],
                        in_=stage[:, 0:(NCHUNK - 1) * CHUNK_N])

            # SWDGE on the (mostly idle) Pool engine so this DMA, which waits
            # on conv2, never head-of-line-blocks the SP x-load stream
            if last_img:
                nc.gpsimd.dma_start(
                    out=out_comp[:, (NCHUNK - 1) * CHUNK_N:S],
                    in_=stage[:, (NCHUNK - 1) * CHUNK_N:S])
            else:
                nc.gpsimd.dma_start(out=out_comp, in_=stage[:])

    nc.compile()
    _CACHE["nc"] = nc
    return nc


def _fold_params(conv1_w, conv1_b, conv2_w, conv2_b,
                 bn1_gamma, bn1_beta, bn1_mean, bn1_var,
                 bn2_gamma, bn2_beta, bn2_mean, bn2_var):
    s1 = bn1_gamma / np.sqrt(bn1_var + EPS)                     # [256]
    t1 = bn1_beta - bn1_mean * s1                               # [256]
    s2 = bn2_gamma / np.sqrt(bn2_var + EPS)                     # [128]
    # conv1 bias, plus block 0's BN1 +t1 pushed through the 1x1 conv
    # (block 1 applies its t1 directly on the ACT engine; see kernel)
    w1 = conv1_w[:, :, 0, 0]                                    # [128, 256]
    t2 = (bn2_beta - bn2_mean * s2) + s2 * (
        conv1_b + w1[:, 0:128] @ t1[0:128])                     # [128]

    import ml_dtypes
    # w1t[p, b, m] = conv1_w[m, b*128+p]  (lhsT blocks for K=256 contraction)
    w1t = np.ascontiguousarray(
        conv1_w[:, :, 0, 0].T.reshape(2, 128, C_MID).transpose(1, 0, 2)
    ).astype(ml_dtypes.bfloat16)
    # w2t[k, dy*3+dx, m] = conv2_w[m, k, dy, dx]
    w2t = np.ascontiguousarray(
        conv2_w.transpose(1, 2, 3, 0).reshape(128, 9, C_OUT)
    ).astype(ml_dtypes.bfloat16)

    bnp = np.zeros((128, 8), np.float32)
    bnp[:, 0] = s1[0:128]
    bnp[:, 1] = s1[128:256]
    bnp[:, 2] = -t1[0:128] / s1[0:128]   # u for the DVE max-mult form
    bnp[:, 3] = t1[128:256]              # plain bias for the ACT form
    bnp[:, 4] = s2
    bnp[:, 5] = t2
    bnp[0:C_OUT, 6] = conv2_b
    return w1t, w2t, bnp


def _get_runner():
    """Build (once) a jitted shard_map that runs the per-core NEFF on all 8
    cores. Mirrors bass2jax.run_bass_via_pjrt but caches the jit and keeps
    the output-backing zero buffer device-resident across calls."""
    if "runner" in _CACHE:
        return _CACHE["runner"]

    import jax
    from jax.sharding import Mesh, PartitionSpec
    from jax.experimental.shard_map import shard_map
    from concourse import bass2jax, mybir
    from concourse.bass2jax import _bass_exec_p, partition_id_tensor

    nc = _build_program()
    bass2jax.install_neuronx_cc_hook()

    partition_name = (nc.partition_id_tensor.name
                      if nc.partition_id_tensor else None)
    in_names = []
    out_names = []
    out_avals = []
    for alloc in nc.m.functions[0].allocations:
        if not isinstance(alloc, mybir.MemoryLocationSet):
            continue
        name = alloc.memorylocations[0].name
        if alloc.kind == "ExternalInput":
            if name != partition_name:
                in_names.append(name)
        elif alloc.kind == "ExternalOutput":
            out_names.append(name)
            out_avals.append(jax.core.ShapedArray(tuple(alloc.tensor_shape),
                                                  mybir.dt.np(alloc.dtype)))
    all_in_names = list(in_names) + list(out_names)
    if partition_name is not None:
        all_in_names.append(partition_name)

    def _body(*args):
        operands = list(args)
        if partition_name is not None:
            operands.append(partition_id_tensor())
        outs = _bass_exec_p.bind(
            *operands,
            out_avals=tuple(out_avals),
            in_names=tuple(all_in_names),
            out_names=tuple(out_names),
            lowering_input_output_aliases=(),
            sim_require_finite=True,
            sim_require_nnan=True,
            nc=nc,
        )
        return outs[0]

    devices = jax.devices()[:N_CORES]
    mesh = Mesh(np.asarray(devices), ("core",))
    nargs = len(in_names) + 1  # + output-backing buffer
    jitted = jax.jit(shard_map(_body, mesh=mesh,
                               in_specs=(PartitionSpec("core"),) * nargs,
                               out_specs=PartitionSpec("core"),
                               check_rep=False))
    zout = jax.device_put(
        np.zeros((N_CORES * N_PER_CORE, C_OUT, H, W), np.float32))
    _CACHE["runner"] = (jitted, list(in_names), zout)
    return _CACHE["runner"]


def kernel(x, bn1_gamma, bn1_beta, bn1_mean, bn1_var, conv1_w, conv1_b,
           bn2_gamma, bn2_beta, bn2_mean, bn2_var, conv2_w, conv2_b):
    x = np.ascontiguousarray(np.asarray(x, dtype=np.float32))
    w1t, w2t, bnp = _fold_params(
        np.asarray(conv1_w, np.float32), np.asarray(conv1_b, np.float32),
        np.asarray(conv2_w, np.float32), np.asarray(conv2_b, np.float32),
        np.asarray(bn1_gamma, np.float32), np.asarray(bn1_beta, np.float32),
        np.asarray(bn1_mean, np.float32), np.asarray(bn1_var, np.float32),
        np.asarray(bn2_gamma, np.float32), np.asarray(bn2_beta, np.float32),
        np.asarray(bn2_mean, np.float32), np.asarray(bn2_var, np.float32))

    jitted, in_names, zout = _get_runner()
    # global arrays: shard axis 0 across the 8 cores (params replicated)
    per_name = {
        "x": x,  # [32, ...] -> [4, ...] per core
        "w1t": np.concatenate([w1t] * N_CORES, axis=0),
        "w2t": np.concatenate([w2t] * N_CORES, axis=0),
        "bnp": np.concatenate([bnp] * N_CORES, axis=0),
    }
    args = [per_name[nm] for nm in in_names]
    comp = jitted(*args, zout)
    # unshard/gather: channels 0..255 of the concat are x verbatim and are
    # assembled here from the host-resident input; the device computes and
    # returns only the 32 conv output channels
    full = np.empty((N_CORES * N_PER_CORE, C_IN + C_OUT, H, W), np.float32)
    full[:, :C_IN] = x
    full[:, C_IN:] = np.asarray(comp)
    return full
